# revision 4
# baseline (speedup 1.0000x reference)
"""Trainium2 Bass kernel for nn_DiffusionModel1d (batched 1-D diffusion solve).

Math: the reference solves A(K) u = f per batch row with K = exp(x) via the
Thomas algorithm, where A = G^T diag(K_hat) G, G the n x n lower-bidiagonal
difference matrix (1 on diag, -1 on subdiag) and
K_hat = (2*K_0, K_1, ..., K_{n-1}).  Hence

    u = h2 * G^{-1} diag(K_hat)^{-1} G^{-T} f
      = h2 * cumsum_j( w_j * exp(-x_j) ),   w = suffix_sum(f), w_0 halved.

So the whole solve is: one exp, one elementwise multiply by a shared
per-column vector, and one hardware prefix-sum scan along the grid dim.
Pure data parallel over batch: 8192 rows -> 1024 rows per core x 8 cores.
"""

import os
import sys

import numpy as np

sys.path.insert(0, "/opt/trn_rl_repo")

import concourse.bacc as bacc
import concourse.mybir as mybir
import concourse.tile as tile
from concourse import bass_utils

B, M = 8192, 2048
N = M - 1
NCORES = 8
BC = B // NCORES          # 1024 batch rows per core
P = 128                   # SBUF partitions
GROUPS = BC // P          # 8 partition-groups per core
H2 = (1.0 / N) ** 2

_cached_nc = None
LAST_RESULTS = None


def _build_kernel():
    fp32 = mybir.dt.float32
    nc = bacc.Bacc(
        "TRN2",
        target_bir_lowering=False,
        debug=False,
        enable_asserts=False,
        num_devices=NCORES,
    )
    x_d = nc.dram_tensor("x", (BC, M), fp32, kind="ExternalInput").ap()
    f_d = nc.dram_tensor("f", (N,), fp32, kind="ExternalInput").ap()
    o_d = nc.dram_tensor("out", (BC, N), fp32, kind="ExternalOutput").ap()

    add = mybir.AluOpType.add
    bypass = mybir.AluOpType.bypass

    with tile.TileContext(nc) as tc:
        with (
            tc.tile_pool(name="const", bufs=1) as cpool,
            tc.tile_pool(name="dram", bufs=1, space="DRAM") as dpool,
            tc.tile_pool(name="work", bufs=3) as pool,
        ):
            # ---- shared per-column weights: w = suffix_sum(f) * h2, w[0] /= 2
            fsb = cpool.tile([1, N], fp32, tag="fsb")
            nc.sync.dma_start(out=fsb, in_=f_d[None, :])
            # prefix-sum of reversed f = suffix sums (in reversed order)
            psum = cpool.tile([1, N], fp32, tag="psum")
            frev = fsb[:, ::-1]
            nc.vector.tensor_tensor_scan(
                out=psum, data0=frev, data1=frev, initial=0.0, op0=add, op1=bypass
            )
            # un-reverse + scale by h2
            w1 = cpool.tile([1, N], fp32, tag="w1")
            nc.scalar.mul(out=w1, in_=psum[:, ::-1], mul=float(H2))
            nc.scalar.mul(out=w1[:, 0:1], in_=w1[:, 0:1], mul=0.5)
            # broadcast across the 128 partitions: bounce via DRAM, then a
            # replicating DMA (step-0 leading dim on the DRAM side)
            wdram = dpool.tile([1, N], fp32, tag="wdram")
            nc.sync.dma_start(out=wdram, in_=w1)
            wb = cpool.tile([P, N], fp32, tag="wb")
            nc.sync.dma_start(
                out=wb, in_=wdram.partition_broadcast(P).squeeze(1)
            )

            # ---- per-group pipeline: DMA in -> exp(-x) -> *w -> cumsum -> DMA out
            for g in range(GROUPS):
                rows = slice(g * P, (g + 1) * P)
                xt = pool.tile([P, M], fp32, tag="x")
                nc.sync.dma_start(out=xt, in_=x_d[rows, :])
                et = pool.tile([P, N], fp32, tag="e")
                nc.scalar.activation(
                    out=et,
                    in_=xt[:, :N],
                    func=mybir.ActivationFunctionType.Exp,
                    scale=-1.0,
                )
                vt = pool.tile([P, N], fp32, tag="v")
                # multiply on GPSIMD: DVE is the bottleneck (scans), Pool is idle
                nc.gpsimd.tensor_mul(out=vt, in0=et, in1=wb)
                ut = pool.tile([P, N], fp32, tag="u")
                nc.vector.tensor_tensor_scan(
                    out=ut, data0=vt, data1=vt, initial=0.0, op0=add, op1=bypass
                )
                nc.sync.dma_start(out=o_d[rows, :], in_=ut)

    nc.compile()
    return nc


def _get_nc():
    global _cached_nc
    if _cached_nc is None:
        _cached_nc = _build_kernel()
    return _cached_nc


def kernel(x: np.ndarray, f_rhs: np.ndarray) -> np.ndarray:
    assert x.shape == (B, M) and f_rhs.shape == (N,)
    x = np.ascontiguousarray(x, dtype=np.float32)
    f_rhs = np.ascontiguousarray(f_rhs, dtype=np.float32)
    nc = _get_nc()
    in_maps = [
        {"x": x[c * BC : (c + 1) * BC], "f": f_rhs} for c in range(NCORES)
    ]
    res = bass_utils.run_bass_kernel_spmd(
        nc,
        in_maps,
        core_ids=list(range(NCORES)),
        trace=bool(int(os.environ.get("KERNEL_TRACE", "0"))),
    )
    global LAST_RESULTS
    LAST_RESULTS = res
    out = np.concatenate(
        [res.results[c]["out"] for c in range(NCORES)], axis=0
    ).astype(np.float32)
    return out


# revision 7
# speedup vs baseline: 1.2860x; 1.2860x over previous
"""Trainium2 Bass kernel for nn_DiffusionModel1d (batched 1-D diffusion solve).

Math: the reference solves A(K) u = f per batch row with K = exp(x) via the
Thomas algorithm, where A = G^T diag(K_hat) G, G the n x n lower-bidiagonal
difference matrix (1 on diag, -1 on subdiag) and
K_hat = (2*K_0, K_1, ..., K_{n-1}).  Hence

    u = h2 * G^{-1} diag(K_hat)^{-1} G^{-T} f
      = h2 * cumsum_j( w_j * exp(-x_j) ),   w = suffix_sum(f), w_0 halved.

So the whole solve is: one exp, one elementwise multiply by a shared
per-column vector, and one hardware prefix-sum scan along the grid dim.
Pure data parallel over batch: 8192 rows -> 1024 rows per core x 8 cores.
"""

import os
import sys

import numpy as np

sys.path.insert(0, "/opt/trn_rl_repo")

import concourse.bacc as bacc
import concourse.mybir as mybir
import concourse.tile as tile
from concourse import bass_utils

B, M = 8192, 2048
N = M - 1
NCORES = 8
BC = B // NCORES          # 1024 batch rows per core
P = 128                   # SBUF partitions
GROUPS = BC // P          # 8 partition-groups per core
H2 = (1.0 / N) ** 2
PRESCALE = 4096.0

_cached_nc = None
LAST_RESULTS = None


def _build_kernel():
    fp32 = mybir.dt.float32
    fp16 = mybir.dt.float16
    nc = bacc.Bacc(
        "TRN2",
        target_bir_lowering=False,
        debug=False,
        enable_asserts=False,
        num_devices=NCORES,
    )
    x_d = nc.dram_tensor("x", (BC, M), fp32, kind="ExternalInput").ap()
    f_d = nc.dram_tensor("f", (N,), fp32, kind="ExternalInput").ap()
    o_d = nc.dram_tensor("out", (BC, N), fp32, kind="ExternalOutput").ap()

    add = mybir.AluOpType.add
    bypass = mybir.AluOpType.bypass

    with tile.TileContext(nc) as tc:
        with (
            tc.tile_pool(name="const", bufs=1) as cpool,
            tc.tile_pool(name="dram", bufs=1, space="DRAM") as dpool,
            tc.tile_pool(name="work", bufs=3) as pool,
        ):
            # ---- shared per-column weights: w = suffix_sum(f) * h2, w[0] /= 2
            fsb = cpool.tile([1, N], fp32, tag="fsb")
            nc.sync.dma_start(out=fsb, in_=f_d[None, :])
            # prefix-sum of reversed f = suffix sums (in reversed order)
            psum = cpool.tile([1, N], fp32, tag="psum")
            frev = fsb[:, ::-1]
            nc.vector.tensor_tensor_scan(
                out=psum, data0=frev, data1=frev, initial=0.0, op0=add, op1=bypass
            )
            # un-reverse + scale by h2 * 2^12 (the 2^12 keeps the fp16
            # intermediates comfortably inside fp16 normal range; it is
            # divided back out during the output cast)
            w1 = cpool.tile([1, N], fp32, tag="w1")
            nc.scalar.mul(out=w1, in_=psum[:, ::-1], mul=float(H2 * PRESCALE))
            nc.scalar.mul(out=w1[:, 0:1], in_=w1[:, 0:1], mul=0.5)
            # broadcast across the 128 partitions: bounce via DRAM, then a
            # replicating DMA (step-0 leading dim on the DRAM side)
            wdram = dpool.tile([1, N], fp32, tag="wdram")
            nc.sync.dma_start(out=wdram, in_=w1)
            wb = cpool.tile([P, N], fp32, tag="wb")
            nc.sync.dma_start(
                out=wb, in_=wdram.partition_broadcast(P).squeeze(1)
            )

            # ---- per-group pipeline: DMA in -> exp(-x) -> *w -> cumsum -> DMA out
            for g in range(GROUPS):
                rows = slice(g * P, (g + 1) * P)
                xt = pool.tile([P, M], fp32, tag="x")
                nc.sync.dma_start(out=xt, in_=x_d[rows, :])
                et = pool.tile([P, N], fp32, tag="e")
                nc.scalar.activation(
                    out=et,
                    in_=xt[:, :N],
                    func=mybir.ActivationFunctionType.Exp,
                    scale=-1.0,
                )
                # fp16 scan operands -> DVE 2x perf mode (fp32 scan state)
                vt = pool.tile([P, N], fp16, tag="v")
                nc.vector.tensor_mul(out=vt, in0=et, in1=wb)
                ut = pool.tile([P, N], fp16, tag="u")
                nc.vector.tensor_tensor_scan(
                    out=ut, data0=vt, data1=vt, initial=0.0, op0=add, op1=bypass
                )
                # cast back to fp32 (and undo the 2^12 prescale) on ACT
                u32 = pool.tile([P, N], fp32, tag="u32")
                nc.scalar.mul(out=u32, in_=ut, mul=float(1.0 / PRESCALE))
                nc.sync.dma_start(out=o_d[rows, :], in_=u32)

    nc.compile()
    return nc


def _get_nc():
    global _cached_nc
    if _cached_nc is None:
        _cached_nc = _build_kernel()
    return _cached_nc


def kernel(x: np.ndarray, f_rhs: np.ndarray) -> np.ndarray:
    assert x.shape == (B, M) and f_rhs.shape == (N,)
    x = np.ascontiguousarray(x, dtype=np.float32)
    f_rhs = np.ascontiguousarray(f_rhs, dtype=np.float32)
    nc = _get_nc()
    in_maps = [
        {"x": x[c * BC : (c + 1) * BC], "f": f_rhs} for c in range(NCORES)
    ]
    res = bass_utils.run_bass_kernel_spmd(
        nc,
        in_maps,
        core_ids=list(range(NCORES)),
        trace=bool(int(os.environ.get("KERNEL_TRACE", "0"))),
    )
    global LAST_RESULTS
    LAST_RESULTS = res
    out = np.concatenate(
        [res.results[c]["out"] for c in range(NCORES)], axis=0
    ).astype(np.float32)
    return out


# revision 8
# speedup vs baseline: 1.5171x; 1.1797x over previous
"""Trainium2 Bass kernel for nn_DiffusionModel1d (batched 1-D diffusion solve).

Math: the reference solves A(K) u = f per batch row with K = exp(x) via the
Thomas algorithm, where A = G^T diag(K_hat) G, G the n x n lower-bidiagonal
difference matrix (1 on diag, -1 on subdiag) and
K_hat = (2*K_0, K_1, ..., K_{n-1}).  Hence

    u = h2 * G^{-1} diag(K_hat)^{-1} G^{-T} f
      = h2 * cumsum_j( w_j * exp(-x_j) ),   w = suffix_sum(f), w_0 halved.

So the whole solve is: one exp, one elementwise multiply by a shared
per-column vector, and one hardware prefix-sum scan along the grid dim.
Pure data parallel over batch: 8192 rows -> 1024 rows per core x 8 cores.

Engine budget per core (measured): DVE scan 4.4us + bf16 mult 1.2us per
128-row group (x8), ACT exp, DMA 16.8 MB at ~350 GB/s.  e/w/v are bf16
(DVE 2x mult mode); the scan state and output stay fp32.
"""

import os
import sys

import numpy as np

sys.path.insert(0, "/opt/trn_rl_repo")

import concourse.bacc as bacc
import concourse.mybir as mybir
import concourse.tile as tile
from concourse import bass_utils

B, M = 8192, 2048
N = M - 1
NCORES = 8
BC = B // NCORES          # 1024 batch rows per core
P = 128                   # SBUF partitions
GROUPS = BC // P          # 8 partition-groups per core
H2 = (1.0 / N) ** 2

_cached_nc = None
LAST_RESULTS = None


def _build_kernel():
    fp32 = mybir.dt.float32
    bf16 = mybir.dt.bfloat16
    nc = bacc.Bacc(
        "TRN2",
        target_bir_lowering=False,
        debug=False,
        enable_asserts=False,
        num_devices=NCORES,
    )
    x_d = nc.dram_tensor("x", (BC, M), fp32, kind="ExternalInput").ap()
    f_d = nc.dram_tensor("f", (N,), fp32, kind="ExternalInput").ap()
    o_d = nc.dram_tensor("out", (BC, N), fp32, kind="ExternalOutput").ap()

    add = mybir.AluOpType.add
    bypass = mybir.AluOpType.bypass

    with tile.TileContext(nc) as tc:
        with (
            tc.tile_pool(name="const", bufs=1) as cpool,
            tc.tile_pool(name="psum", bufs=1, space="PSUM") as ppool,
            tc.tile_pool(name="work", bufs=4) as pool,
        ):
            # ---- shared per-column weights: w = suffix_sum(f) * h2, w[0] /= 2
            fsb = cpool.tile([1, N], fp32, tag="fsb")
            nc.sync.dma_start(out=fsb, in_=f_d[None, :])
            # prefix-sum of reversed f = suffix sums (in reversed order)
            psumt = cpool.tile([1, N], fp32, tag="psumt")
            frev = fsb[:, ::-1]
            nc.vector.tensor_tensor_scan(
                out=psumt, data0=frev, data1=frev, initial=0.0, op0=add, op1=bypass
            )
            # un-reverse + scale by h2, downcast to bf16
            w1 = cpool.tile([1, N], bf16, tag="w1")
            nc.scalar.mul(out=w1, in_=psumt[:, ::-1], mul=float(H2))
            nc.scalar.mul(out=w1[:, 0:1], in_=w1[:, 0:1], mul=0.5)
            # broadcast w across the 128 partitions on the tensor engine:
            # ones[1,128]^T @ w1[1,:] -> PSUM, then one ACT copy -> SBUF bf16
            ones = cpool.tile([1, P], bf16, tag="ones")
            nc.vector.memset(ones, 1.0)
            wb = cpool.tile([P, N], bf16, tag="wb")
            for c0 in range(0, N, 512):
                c1 = min(c0 + 512, N)
                wp = ppool.tile([P, 512], fp32, tag="wp")
                nc.tensor.matmul(
                    wp[:, : c1 - c0], ones, w1[:, c0:c1], start=True, stop=True
                )
                nc.scalar.copy(out=wb[:, c0:c1], in_=wp[:, : c1 - c0])

            # ---- per-group pipeline: DMA in -> exp(-x) -> *w -> cumsum -> DMA out
            for g in range(GROUPS):
                rows = slice(g * P, (g + 1) * P)
                xt = pool.tile([P, M], fp32, tag="x")
                nc.sync.dma_start(out=xt, in_=x_d[rows, :])
                et = pool.tile([P, N], bf16, tag="e")
                nc.scalar.activation(
                    out=et,
                    in_=xt[:, :N],
                    func=mybir.ActivationFunctionType.Exp,
                    scale=-1.0,
                )
                # bf16 x bf16 -> bf16 multiply runs in the DVE 2x perf mode
                vt = pool.tile([P, N], bf16, tag="v")
                nc.vector.tensor_mul(out=vt, in0=et, in1=wb)
                # prefix sum along the grid dim; fp32 state and fp32 output
                ut = pool.tile([P, N], fp32, tag="u")
                nc.vector.tensor_tensor_scan(
                    out=ut, data0=vt, data1=vt, initial=0.0, op0=add, op1=bypass
                )
                nc.sync.dma_start(out=o_d[rows, :], in_=ut)

    nc.compile()
    return nc


def _get_nc():
    global _cached_nc
    if _cached_nc is None:
        _cached_nc = _build_kernel()
    return _cached_nc


def kernel(x: np.ndarray, f_rhs: np.ndarray) -> np.ndarray:
    assert x.shape == (B, M) and f_rhs.shape == (N,)
    x = np.ascontiguousarray(x, dtype=np.float32)
    f_rhs = np.ascontiguousarray(f_rhs, dtype=np.float32)
    nc = _get_nc()
    in_maps = [
        {"x": x[c * BC : (c + 1) * BC], "f": f_rhs} for c in range(NCORES)
    ]
    res = bass_utils.run_bass_kernel_spmd(
        nc,
        in_maps,
        core_ids=list(range(NCORES)),
        trace=bool(int(os.environ.get("KERNEL_TRACE", "0"))),
    )
    global LAST_RESULTS
    LAST_RESULTS = res
    out = np.concatenate(
        [res.results[c]["out"] for c in range(NCORES)], axis=0
    ).astype(np.float32)
    return out
